# revision 1
# baseline (speedup 1.0000x reference)
"""Luong attention kernel for Trainium2 (Bass/Tile), data-parallel over batch.

Math (per batch b):
    scores[s,t] = enc[s,:] . dec[t,:]
    weights     = softmax(scores, axis=t)
    context[s]  = sum_t weights[s,t] * enc[t,:]
    out         = tanh(concat([context, dec]) @ W_tanh)

Implementation notes:
  - B=8 batches -> 8 NeuronCores, one batch per core, no collectives.
  - scoresT[t,s] is computed (t on partitions) so the context contraction
    over t maps directly onto the PE (lhsT = enc natural, rhs = exp(scoresT)).
  - softmax uses a *global* shift (softmax is shift-invariant): E = exp(s-64).
    Scores ~ N(0, 256): row max is ~[45..95], so exp(s-64) stays inside
    fp32/bf16 range on both ends; E is kept unnormalized and the
    normalization (1/denom) is applied after the final matmul, where denom
    is per output row s (a per-partition scalar there).
  - denom[s] = sum_t E[t,s] is accumulated chunkwise on DVE (Esum) and the
    final cross-partition fold uses 16 tiny PE matmuls with a ones vector,
    which lands denom directly in [s-partition, 1] layout.
  - All matmul operands are bf16 (full PE rate); accumulation is fp32 PSUM.
"""

import sys

if "/opt/trn_rl_repo" not in sys.path:
    sys.path.insert(0, "/opt/trn_rl_repo")

import numpy as np

import concourse.bacc as bacc
import concourse.mybir as mybir
import concourse.tile as tile
from concourse import bass_utils

B, S, D = 8, 2048, 256
P = 128
NT = S // P  # 16 chunks of 128 along t (and s for output rows)
SB = 512  # moving-dim block for the big matmuls
NSB = S // SB  # 4
DC = D // P  # 2 partition chunks of the feature dim
SHIFT = 64.0  # global softmax shift

_CACHE = {}


def _build(reps: int = 1):
    f32, bf16, f16 = mybir.dt.float32, mybir.dt.bfloat16, mybir.dt.float16
    AF = mybir.ActivationFunctionType

    nc = bacc.Bacc("TRN2", target_bir_lowering=False, debug=False)
    enc_d = nc.dram_tensor("enc", [S, D], f32, kind="ExternalInput").ap()
    dec_d = nc.dram_tensor("dec", [S, D], f32, kind="ExternalInput").ap()
    w_d = nc.dram_tensor("w", [2 * D, D], f32, kind="ExternalInput").ap()
    out_d = nc.dram_tensor("out", [S, D], f32, kind="ExternalOutput").ap()

    with tile.TileContext(nc) as tc:
        with (
            tc.tile_pool(name="big", bufs=1) as big,
            tc.tile_pool(name="stage", bufs=1) as stage,
        ):
            encT = big.tile([P, DC, S], f16, tag="encT")  # enc^T  (d-part, s-free)
            decT = big.tile([P, DC, S], f16, tag="decT")  # dec^T
            encN = big.tile([P, NT, D], bf16, tag="encN")  # enc natural, per t-chunk
            E = big.tile([P, NT, S], bf16, tag="E")  # exp(scoresT - SHIFT)
            Esum = big.tile([P, S], f32, tag="Esum")  # partial denom (128-fold)
            EsumB = big.tile([P, S], bf16, tag="EsumB")
            U = big.tile([P, DC, S], bf16, tag="U")  # unnormalized context^T
            Wt1 = big.tile([P, DC, D], bf16, tag="Wt1")  # W_tanh rows 0..255 (ctx)
            Wt2 = big.tile([P, DC, D], f16, tag="Wt2")  # W_tanh rows 256..511 (dec)
            ones = big.tile([P, 1], bf16, tag="ones")
            rden = big.tile([P, NT], f32, tag="rden")  # 1/denom, [s-part, s-chunk]
            nshift = big.tile([P, 1], f32, tag="nshift")
            zbias = big.tile([P, 1], f32, tag="zbias")

            outS = big.tile([P, NT, D], f32, tag="outS")  # staged output rows

            nc.any.memset(ones[:], 1.0)
            nc.any.memset(nshift[:], -SHIFT)
            nc.any.memset(zbias[:], 0.0)

            for _rep in range(reps):
                _body(nc, tc, big, stage, locals())

    nc.compile()
    return nc


def _body(nc, tc, big, stage, env):
    f32, bf16, f16 = mybir.dt.float32, mybir.dt.bfloat16, mybir.dt.float16
    AF = mybir.ActivationFunctionType
    enc_d, dec_d, w_d, out_d = env["enc_d"], env["dec_d"], env["w_d"], env["out_d"]
    encT, decT, encN, E = env["encT"], env["decT"], env["encN"], env["E"]
    Esum, EsumB, U = env["Esum"], env["EsumB"], env["U"]
    Wt1, Wt2, ones, rden = env["Wt1"], env["Wt2"], env["ones"], env["rden"]
    nshift, zbias, outS = env["nshift"], env["zbias"], env["outS"]

    if True:
        if True:
            # ---- transposed operands: cast to f16, bounce via DRAM scratch,
            # then one big DMA-transpose per 128-row half (xbar is 16-bit only).
            with tc.tile_pool(name="scr", bufs=1, space="DRAM") as scr:
                encS = stage.tile([P, NT, D], f32, tag="encS")
                decS = stage.tile([P, NT, D], f32, tag="decS")
                encH = stage.tile([P, NT, D], f16, tag="encH")
                decH = stage.tile([P, NT, D], f16, tag="decH")
                scrE = scr.tile([S, D], f16, tag="scrE")
                scrD = scr.tile([S, D], f16, tag="scrD")

                nc.sync.dma_start(decS[:], dec_d.rearrange("(n p) d -> p n d", p=P))
                nc.sync.dma_start(encS[:], enc_d.rearrange("(n p) d -> p n d", p=P))
                nc.vector.tensor_copy(decH[:], decS[:])
                nc.vector.tensor_copy(encH[:], encS[:])
                nc.vector.tensor_copy(encN[:], encS[:])
                nc.sync.dma_start(scrD.rearrange("(n p) d -> p n d", p=P), decH[:])
                nc.sync.dma_start(scrE.rearrange("(n p) d -> p n d", p=P), encH[:])
                for src, dsth in ((scrD, decT), (scrE, encT)):
                    for dc in range(DC):
                        nc.sync.dma_start(
                            out=dsth[:, dc, :],
                            in_=src[:, dc * P : (dc + 1) * P],
                            transpose=True,
                        )

            # ---- W: one batched DMA; rows 0..255 -> bf16 (ctx), 256..511 -> f16
            wst = stage.tile([P, 4, D], f32, tag="wst")
            nc.sync.dma_start(wst[:], w_d.rearrange("(r p) d -> p r d", p=P))
            for r in range(2):
                nc.vector.tensor_copy(Wt1[:, r, :], wst[:, r, :])
                nc.vector.tensor_copy(Wt2[:, r, :], wst[:, 2 + r, :])

            # ---- fused phases 1+2, s-block outer: scores->exp->E for one
            # s-block, then that block's U accumulation; U(sb) overlaps
            # scores(sb+1) with no global barrier.
            with (
                tc.tile_pool(name="ps_s", bufs=3, space="PSUM") as ps_s,
                tc.tile_pool(name="ps_u", bufs=4, space="PSUM") as ps_u,
            ):
                for sb in range(NSB):
                    s_lo, s_hi = sb * SB, (sb + 1) * SB
                    for t in range(NT):
                        ps = ps_s.tile([P, SB], f32, tag="ps")
                        for dc in range(DC):
                            nc.tensor.matmul(
                                ps[:],
                                decT[:, dc, t * P : (t + 1) * P],
                                encT[:, dc, s_lo:s_hi],
                                start=(dc == 0),
                                stop=(dc == DC - 1),
                            )
                        nc.scalar.activation(
                            E[:, t, s_lo:s_hi], ps[:], AF.Exp, bias=nshift[:]
                        )
                        if t == 0:
                            nc.vector.tensor_copy(
                                Esum[:, s_lo:s_hi], E[:, t, s_lo:s_hi]
                            )
                        else:
                            nc.vector.tensor_add(
                                Esum[:, s_lo:s_hi],
                                Esum[:, s_lo:s_hi],
                                E[:, t, s_lo:s_hi],
                            )
                    for dc in range(DC):
                        pu = ps_u.tile([P, SB], f32, tag="pu")
                        for t in range(NT):
                            nc.tensor.matmul(
                                pu[:],
                                encN[:, t, dc * P : (dc + 1) * P],
                                E[:, t, s_lo:s_hi],
                                start=(t == 0),
                                stop=(t == NT - 1),
                            )
                        nc.vector.tensor_copy(U[:, dc, s_lo:s_hi], pu[:])

            # ---- denominator: fold Esum across partitions, then reciprocal
            nc.vector.tensor_copy(EsumB[:], Esum[:])
            with tc.tile_pool(name="ps_d", bufs=1, space="PSUM") as ps_d:
                pd = ps_d.tile([P, NT], f32, tag="pd")
                for c in range(NT):
                    nc.tensor.matmul(
                        pd[:, c : c + 1],
                        EsumB[:, c * P : (c + 1) * P],
                        ones[:],
                        start=True,
                        stop=True,
                    )
                nc.vector.reciprocal(rden[:], pd[:])

            # ---- phase 3: out = tanh(U^T@W1 / denom + dec@W2)
            with (
                tc.tile_pool(name="ps_y", bufs=2, space="PSUM") as ps_y,
                tc.tile_pool(name="fout", bufs=3) as fout,
            ):
                for c in range(NT):
                    y1 = ps_y.tile([P, D], f32, tag="y1")
                    y2 = ps_y.tile([P, D], f32, tag="y2")
                    for dc in range(DC):
                        nc.tensor.matmul(
                            y1[:],
                            U[:, dc, c * P : (c + 1) * P],
                            Wt1[:, dc, :],
                            start=(dc == 0),
                            stop=(dc == DC - 1),
                        )
                    for dc in range(DC):
                        nc.tensor.matmul(
                            y2[:],
                            decT[:, dc, c * P : (c + 1) * P],
                            Wt2[:, dc, :],
                            start=(dc == 0),
                            stop=(dc == DC - 1),
                        )
                    t1 = fout.tile([P, D], f32, tag="t1")
                    nc.vector.tensor_scalar_mul(t1[:], y1[:], rden[:, c : c + 1])
                    t2 = fout.tile([P, D], f32, tag="t2")
                    nc.vector.tensor_add(t2[:], t1[:], y2[:])
                    nc.scalar.activation(outS[:, c, :], t2[:], AF.Tanh, bias=zbias[:])
                nc.sync.dma_start(
                    out_d.rearrange("(n p) d -> p n d", p=P), outS[:]
                )


def get_nc():
    if "nc" not in _CACHE:
        _CACHE["nc"] = _build()
    return _CACHE["nc"]


def _get_fn():
    """Build the sharded PJRT executable once and cache it; subsequent
    kernel() calls pay only input transfer + dispatch."""
    if "fn" in _CACHE:
        return _CACHE["fn"]
    import jax
    from jax.sharding import Mesh, NamedSharding, PartitionSpec
    from jax.experimental.shard_map import shard_map
    from concourse.bass2jax import (
        _bass_exec_p,
        install_neuronx_cc_hook,
        partition_id_tensor,
    )

    install_neuronx_cc_hook()
    nc = get_nc()
    out_avals = []
    for alloc in nc.m.functions[0].allocations:
        if (
            isinstance(alloc, mybir.MemoryLocationSet)
            and alloc.kind == "ExternalOutput"
        ):
            out_avals.append(
                jax.core.ShapedArray(
                    tuple(alloc.tensor_shape), mybir.dt.np(alloc.dtype)
                )
            )
    has_pid = nc.partition_id_tensor is not None
    names = ["enc", "dec", "w", "out"] + (["partition_id"] if has_pid else [])
    mesh = Mesh(np.asarray(jax.devices()[:B]), ("core",))
    spec = PartitionSpec("core")

    def _b(e, d, ww, z):
        ops = [e, d, ww, z] + ([partition_id_tensor()] if has_pid else [])
        return _bass_exec_p.bind(
            *ops,
            out_avals=tuple(out_avals),
            in_names=tuple(names),
            out_names=("out",),
            lowering_input_output_aliases=(),
            sim_require_finite=True,
            sim_require_nnan=True,
            nc=nc,
        )[0]

    jitted = jax.jit(
        shard_map(
            _b, mesh=mesh, in_specs=(spec,) * 4, out_specs=spec, check_rep=False
        ),
        donate_argnums=(3,),
        keep_unused=True,
    )
    sh = NamedSharding(mesh, spec)
    _CACHE["fn"] = (jitted, sh)
    return _CACHE["fn"]


def kernel(enc_outputs_top, dec_outputs_top, W_tanh):
    import jax

    enc = np.ascontiguousarray(enc_outputs_top, dtype=np.float32)
    dec = np.ascontiguousarray(dec_outputs_top, dtype=np.float32)
    w = np.ascontiguousarray(W_tanh, dtype=np.float32)
    try:
        fn, sh = _get_fn()
        eg = jax.device_put(enc.reshape(B * S, D), sh)
        dg = jax.device_put(dec.reshape(B * S, D), sh)
        wg = jax.device_put(np.concatenate([w] * B, axis=0), sh)
        zg = jax.device_put(np.zeros((B * S, D), np.float32), sh)
        out = np.asarray(jax.block_until_ready(fn(eg, dg, wg, zg)))
        return out.reshape(B, S, D)
    except Exception:
        # fallback: reference multi-core path (rebuilds the jit per call)
        nc = get_nc()
        in_maps = [{"enc": enc[b], "dec": dec[b], "w": w} for b in range(B)]
        res = bass_utils.run_bass_kernel_spmd(nc, in_maps, core_ids=list(range(B)))
        return np.stack([r["out"] for r in res.results], axis=0)



# revision 3
# speedup vs baseline: 2.8084x; 2.8084x over previous
"""Luong attention kernel for Trainium2 (Bass/Tile), data-parallel over batch.

Math (per batch b):
    scores[s,t] = enc[s,:] . dec[t,:]
    weights     = softmax(scores, axis=t)
    context[s]  = sum_t weights[s,t] * enc[t,:]
    out         = tanh(concat([context, dec]) @ W_tanh)

v2 design notes (vs the v1 baseline):
  - B=8 batches -> 8 NeuronCores, one batch per core, no collectives.
  - Input staging is PREFETCHED one rep ahead (all stage tiles double
    buffered), so the load->cast->bounce->transpose chain of rep r+1
    overlaps rep r's compute and the PE never idles at rep boundaries
    (keeps the HAM clock-gate warm).
  - f32->f16/bf16 casts ride the DMA (SWDGE cast DMAs on gpsimd)
    instead of burning DVE cycles.
  - Phases 1+2 are software-pipelined at t-chunk granularity: the PE
    stream per t-slot is [2 scores MMs (t)] [2 context MMs (t-1)], so
    context matmuls never wait on the Exp activation (ACT is ~2 chunks
    behind but its 611ns/chunk hides under the 864ns of 4 MMs).
  - softmax uses a *global* shift (softmax is shift-invariant): E =
    exp(s-64), kept unnormalized in bf16; normalization (1/denom) is
    applied after the final matmul where denom is per output row s.
  - denom partials (fold over the 16 t-chunks) are a 4-level batched
    bf16 tree on DVE (17us instead of 44us of fp32 serial adds), the
    cross-partition 128-fold is 4 tiny PE matmuls per block with a ones
    vector, placed in the PE stream half a block later so the PE never
    waits for the tree.
  - All matmul operands are 16-bit (full PE rate); accumulation is fp32
    PSUM.  PSUM budget: 3 (scores) + 1 (pd) + 2 (context) + 2 (y1/y2)
    = 8 banks.
  - Staging tiles are TRIPLE buffered: with bufs=2 the next rep's cast
    DMAs carry WAR waits on PE matmuls deep into the current rep (the
    slot is only freed by the last reader), which delayed the scr
    bounce -> transposes -> first scores matmuls of the next rep by
    ~5us.  bufs=3 makes those waits trivially satisfied.
  - Phase-3 y1/y2 share one PSUM bank as a single 4-matmul accumulation
    group (start clears the whole bank's has_written bits; y2's fresh
    region overwrites), and each block's 4 denominator folds are one
    group: fewer stop boundaries -> fewer semaphore round trips.
"""

import sys

if "/opt/trn_rl_repo" not in sys.path:
    sys.path.insert(0, "/opt/trn_rl_repo")

import numpy as np

import concourse.bacc as bacc
import concourse.mybir as mybir
import concourse.tile as tile
from concourse import bass_utils

B, S, D = 8, 2048, 256
P = 128
NT = S // P  # 16 chunks of 128 along t (and s for output rows)
SB = 512  # moving-dim block for the big matmuls
NSB = S // SB  # 4
DC = D // P  # 2 partition chunks of the feature dim
SHIFT = 64.0  # global softmax shift

_CACHE = {}


def _build(reps: int = 1):
    f32, bf16, f16 = mybir.dt.float32, mybir.dt.bfloat16, mybir.dt.float16
    AF = mybir.ActivationFunctionType

    nc = bacc.Bacc("TRN2", target_bir_lowering=False, debug=False)
    enc_d = nc.dram_tensor("enc", [S, D], f32, kind="ExternalInput").ap()
    dec_d = nc.dram_tensor("dec", [S, D], f32, kind="ExternalInput").ap()
    w_d = nc.dram_tensor("w", [2 * D, D], f32, kind="ExternalInput").ap()
    out_d = nc.dram_tensor("out", [S, D], f32, kind="ExternalOutput").ap()
    out_v = out_d.rearrange("(n p) d -> p n d", p=P)

    with tile.TileContext(nc) as tc:
        with (
            tc.tile_pool(name="pers", bufs=1) as pers,
            tc.tile_pool(name="stg", bufs=3) as stg,
            tc.tile_pool(name="scr", bufs=2, space="DRAM") as scr,
            tc.tile_pool(name="wrk", bufs=2) as wrk,
            tc.tile_pool(name="psA", bufs=3, space="PSUM") as psA,
            tc.tile_pool(name="psU", bufs=2, space="PSUM") as psU,
            tc.tile_pool(name="psY", bufs=2, space="PSUM") as psY,
        ):
            ones = pers.tile([P, 1], bf16, tag="ones")
            nshift = pers.tile([P, 1], f32, tag="nshift")
            nc.any.memset(ones[:], 1.0)
            nc.any.memset(nshift[:], -SHIFT)

            def stage_casts():
                """Input-staging part 1: cast DMAs (no intra-stage waits on
                the Pool queue: encN/Wt read the external inputs directly)."""
                scrE = scr.tile([S, D], f16, tag="scrE")
                scrD = scr.tile([S, D], f16, tag="scrD")
                encN = stg.tile([P, NT, D], bf16, tag="encN")
                Wt1 = stg.tile([P, DC, D], bf16, tag="Wt1")
                Wt2 = stg.tile([P, DC, D], f16, tag="Wt2")
                nc.gpsimd.dma_start(
                    encN[:], enc_d.rearrange("(n p) d -> p n d", p=P)
                )
                nc.gpsimd.dma_start(
                    Wt1[:], w_d[0:D, :].rearrange("(r p) d -> p r d", p=P)
                )
                nc.gpsimd.dma_start(
                    Wt2[:], w_d[D : 2 * D, :].rearrange("(r p) d -> p r d", p=P)
                )
                nc.gpsimd.dma_start(scrE[:], enc_d[:])
                nc.gpsimd.dma_start(scrD[:], dec_d[:])
                return dict(scrE=scrE, scrD=scrD, encN=encN, Wt1=Wt1, Wt2=Wt2)

            def stage_transposes(g):
                """Input-staging part 2: xbar transposes (emitted mid-body so
                the SP queue position neither delays the out stores nor the
                next rep's first scores matmuls)."""
                encT = stg.tile([P, DC, S], f16, tag="encT")
                decT = stg.tile([P, DC, S], f16, tag="decT")
                for src, dst in ((g["scrE"], encT), (g["scrD"], decT)):
                    for dc in range(DC):
                        nc.sync.dma_start(
                            out=dst[:, dc, :],
                            in_=src[:, dc * P : (dc + 1) * P],
                            transpose=True,
                        )
                g["encT"] = encT
                g["decT"] = decT

            def body(g):
                encT, decT, encN = g["encT"], g["decT"], g["encN"]
                Wt1, Wt2 = g["Wt1"], g["Wt2"]

                U = wrk.tile([P, DC, S], bf16, tag="U")
                rden = wrk.tile([P, NT], f32, tag="rden")
                pd = psA.tile([P, NT], f32, tag="pd", bufs=1)
                esumBs = [None] * NSB

                def emit_fold(b):
                    # cross-partition fold of block b's Esum + reciprocal
                    eb = esumBs[b]
                    for j in range(4):
                        c = b * 4 + j
                        nc.tensor.matmul(
                            pd[:, c : c + 1],
                            eb[:, j * P : (j + 1) * P],
                            ones[:],
                            start=(j == 0),
                            stop=(j == 3),
                        )
                    nc.vector.reciprocal(
                        rden[:, b * 4 : (b + 1) * 4], pd[:, b * 4 : (b + 1) * 4]
                    )

                # ---- phases 1+2, software-pipelined per t-chunk
                for b in range(NSB):
                    s_lo = b * SB
                    E = wrk.tile([P, NT, SB], bf16, tag="E")
                    T = wrk.tile([P, 14, SB], bf16, tag="T")
                    pu = [
                        psU.tile([P, SB], f32, tag="pu", name=f"pu{b}_{i}")
                        for i in range(DC)
                    ]

                    def ctx_mms(t):
                        for dc in range(DC):
                            nc.tensor.matmul(
                                pu[dc][:],
                                encN[:, t, dc * P : (dc + 1) * P],
                                E[:, t, :],
                                start=(t == 0),
                                stop=(t == NT - 1),
                            )

                    for t in range(NT):
                        ps = psA.tile([P, SB], f32, tag="ps")
                        for dc in range(DC):
                            nc.tensor.matmul(
                                ps[:],
                                decT[:, dc, t * P : (t + 1) * P],
                                encT[:, dc, s_lo : s_lo + SB],
                                start=(dc == 0),
                                stop=(dc == DC - 1),
                            )
                        nc.scalar.activation(
                            E[:, t, :], ps[:], AF.Exp, bias=nshift[:]
                        )
                        if t >= 1:
                            ctx_mms(t - 1)
                        if b >= 1 and t == 8:
                            emit_fold(b - 1)
                        if b == 1 and t == 4 and g.get("nxt") is not None:
                            stage_transposes(g["nxt"])
                    ctx_mms(NT - 1)
                    for dc in range(DC):
                        nc.vector.tensor_copy(U[:, dc, s_lo : s_lo + SB], pu[dc][:])
                    # denom partials: 4-level batched bf16 tree over t-chunks
                    eB = wrk.tile([P, SB], bf16, tag="eB")
                    nc.vector.tensor_add(T[:, 0:8, :], E[:, 0:8, :], E[:, 8:16, :])
                    nc.vector.tensor_add(T[:, 8:12, :], T[:, 0:4, :], T[:, 4:8, :])
                    nc.vector.tensor_add(
                        T[:, 12:14, :], T[:, 8:10, :], T[:, 10:12, :]
                    )
                    nc.vector.tensor_add(eB[:], T[:, 12, :], T[:, 13, :])
                    esumBs[b] = eB

                # ---- phase 3: out = tanh(U^T@W1 / denom + dec@W2)
                outO = None
                for c in range(NT):
                    if c == 0:
                        emit_fold(NSB - 1)
                    if c % 4 == 0:
                        outO = wrk.tile([P, 4, D], f32, tag="outO")
                    Y = psY.tile([P, 2 * D], f32, tag="Y")
                    y1 = Y[:, 0:D]
                    y2 = Y[:, D : 2 * D]
                    for dc in range(DC):
                        nc.tensor.matmul(
                            y1,
                            U[:, dc, c * P : (c + 1) * P],
                            Wt1[:, dc, :],
                            start=(dc == 0),
                            stop=False,
                        )
                    for dc in range(DC):
                        nc.tensor.matmul(
                            y2,
                            decT[:, dc, c * P : (c + 1) * P],
                            Wt2[:, dc, :],
                            start=False,
                            stop=(dc == DC - 1),
                        )
                    t1 = wrk.tile([P, D], f32, tag="t1", bufs=4)
                    nc.vector.tensor_scalar_mul(t1[:], y1, rden[:, c : c + 1])
                    t2 = wrk.tile([P, D], f32, tag="t2", bufs=4)
                    nc.vector.tensor_add(t2[:], t1[:], y2)
                    nc.scalar.activation(outO[:, c % 4, :], t2[:], AF.Tanh)
                    if c % 4 == 3:
                        q = c // 4
                        nc.sync.dma_start(out_v[:, q * 4 : (q + 1) * 4, :], outO[:])

            cur = stage_casts()
            stage_transposes(cur)
            for r in range(reps):
                nxt = stage_casts() if r + 1 < reps else None
                cur["nxt"] = nxt
                body(cur)
                cur = nxt

    nc.compile()
    return nc


def get_nc():
    if "nc" not in _CACHE:
        _CACHE["nc"] = _build()
    return _CACHE["nc"]


def _get_fn():
    """Build the sharded PJRT executable once and cache it; subsequent
    kernel() calls pay only input transfer + dispatch."""
    if "fn" in _CACHE:
        return _CACHE["fn"]
    import jax
    from jax.sharding import Mesh, NamedSharding, PartitionSpec
    from jax.experimental.shard_map import shard_map
    from concourse.bass2jax import (
        _bass_exec_p,
        install_neuronx_cc_hook,
        partition_id_tensor,
    )

    install_neuronx_cc_hook()
    nc = get_nc()
    out_avals = []
    for alloc in nc.m.functions[0].allocations:
        if (
            isinstance(alloc, mybir.MemoryLocationSet)
            and alloc.kind == "ExternalOutput"
        ):
            out_avals.append(
                jax.core.ShapedArray(
                    tuple(alloc.tensor_shape), mybir.dt.np(alloc.dtype)
                )
            )
    has_pid = nc.partition_id_tensor is not None
    names = ["enc", "dec", "w", "out"] + (["partition_id"] if has_pid else [])
    mesh = Mesh(np.asarray(jax.devices()[:B]), ("core",))
    spec = PartitionSpec("core")

    def _b(e, d, ww, z):
        ops = [e, d, ww, z] + ([partition_id_tensor()] if has_pid else [])
        return _bass_exec_p.bind(
            *ops,
            out_avals=tuple(out_avals),
            in_names=tuple(names),
            out_names=("out",),
            lowering_input_output_aliases=(),
            sim_require_finite=True,
            sim_require_nnan=True,
            nc=nc,
        )[0]

    jitted = jax.jit(
        shard_map(
            _b, mesh=mesh, in_specs=(spec,) * 4, out_specs=spec, check_rep=False
        ),
        donate_argnums=(3,),
        keep_unused=True,
    )
    sh = NamedSharding(mesh, spec)
    _CACHE["fn"] = (jitted, sh)
    return _CACHE["fn"]


def kernel(enc_outputs_top, dec_outputs_top, W_tanh):
    import jax

    enc = np.ascontiguousarray(enc_outputs_top, dtype=np.float32)
    dec = np.ascontiguousarray(dec_outputs_top, dtype=np.float32)
    w = np.ascontiguousarray(W_tanh, dtype=np.float32)
    try:
        fn, sh = _get_fn()
        eg = jax.device_put(enc.reshape(B * S, D), sh)
        dg = jax.device_put(dec.reshape(B * S, D), sh)
        wg = jax.device_put(np.concatenate([w] * B, axis=0), sh)
        zg = jax.device_put(np.zeros((B * S, D), np.float32), sh)
        out = np.asarray(jax.block_until_ready(fn(eg, dg, wg, zg)))
        return out.reshape(B, S, D)
    except Exception:
        # fallback: reference multi-core path (rebuilds the jit per call)
        nc = get_nc()
        in_maps = [{"enc": enc[b], "dec": dec[b], "w": w} for b in range(B)]
        res = bass_utils.run_bass_kernel_spmd(nc, in_maps, core_ids=list(range(B)))
        return np.stack([r["out"] for r in res.results], axis=0)


# revision 4
# speedup vs baseline: 2.8200x; 1.0041x over previous
"""Luong attention kernel for Trainium2 (Bass/Tile), data-parallel over batch.

Math (per batch b):
    scores[s,t] = enc[s,:] . dec[t,:]
    weights     = softmax(scores, axis=t)
    context[s]  = sum_t weights[s,t] * enc[t,:]
    out         = tanh(concat([context, dec]) @ W_tanh)

v5 design notes:
  - B=8 batches -> 8 NeuronCores, one batch per core, no collectives.
  - Input staging prefetched one rep ahead; staging tiles triple
    buffered so next-rep cast DMAs never wait on PE progress; f32->16bit
    casts ride SWDGE DMAs; transposes via DRAM bounce + xbar.
  - Phases 1+2 software-pipelined per t-chunk: PE slot = [2 scores MMs
    (t), 2 context MMs (t-1)]; the Exp activation (611ns) hides under
    the 864ns of 4 matmuls.
  - softmax via global shift (E = exp(s-64), bf16, unnormalized);
    normalization applied after the final matmul per output row.
  - denominator partials accumulate SERIALLY in bf16 on DVE (one add
    per t-slot, pipelined behind the exps) so each block's partial is
    final one slot after its last exp; the cross-partition 128-fold is
    a single 4-matmul group per block placed half a block later.
  - PHASE 3 OF REP r IS INTERLEAVED INTO REP r+1's BLOCK-0 t-LOOP (one
    output chunk per t-slot): its 4 matmuls fill the PE slack behind
    block-0's own matmuls, removing both the per-chunk DVE-paced PE
    bubbles and the rep-boundary drain.  The last rep runs phase 3 as
    a tail.
  - PSUM: 3 (scores) + 1 (pd) + 2 (context) + 2 (y1/y2) = 8 banks;
    phase-3 y1/y2 share one bank as a single 4-matmul group.
"""

import sys

if "/opt/trn_rl_repo" not in sys.path:
    sys.path.insert(0, "/opt/trn_rl_repo")

import numpy as np

import concourse.bacc as bacc
import concourse.mybir as mybir
import concourse.tile as tile
from concourse import bass_utils

B, S, D = 8, 2048, 256
P = 128
NT = S // P
SB = 512
NSB = S // SB
DC = D // P
SHIFT = 64.0

_CACHE = {}


def _build(reps: int = 1):
    f32, bf16, f16 = mybir.dt.float32, mybir.dt.bfloat16, mybir.dt.float16
    AF = mybir.ActivationFunctionType

    nc = bacc.Bacc("TRN2", target_bir_lowering=False, debug=False)
    enc_d = nc.dram_tensor("enc", [S, D], f32, kind="ExternalInput").ap()
    dec_d = nc.dram_tensor("dec", [S, D], f32, kind="ExternalInput").ap()
    w_d = nc.dram_tensor("w", [2 * D, D], f32, kind="ExternalInput").ap()
    out_d = nc.dram_tensor("out", [S, D], f32, kind="ExternalOutput").ap()
    out_v = out_d.rearrange("(n p) d -> p n d", p=P)

    with tile.TileContext(nc) as tc:
        with (
            tc.tile_pool(name="pers", bufs=1) as pers,
            tc.tile_pool(name="stg", bufs=3) as stg,
            tc.tile_pool(name="scr", bufs=2, space="DRAM") as scr,
            tc.tile_pool(name="wrk", bufs=2) as wrk,
            tc.tile_pool(name="psA", bufs=3, space="PSUM") as psA,
            tc.tile_pool(name="psU", bufs=2, space="PSUM") as psU,
            tc.tile_pool(name="psY", bufs=2, space="PSUM") as psY,
        ):
            ones = pers.tile([P, 1], bf16, tag="ones")
            nshift = pers.tile([P, 1], f32, tag="nshift")
            nc.any.memset(ones[:], 1.0)
            nc.any.memset(nshift[:], -SHIFT)

            def stage_casts():
                scrE = scr.tile([S, D], f16, tag="scrE")
                scrD = scr.tile([S, D], f16, tag="scrD")
                encN = stg.tile([P, NT, D], bf16, tag="encN")
                Wt1 = stg.tile([P, DC, D], bf16, tag="Wt1")
                Wt2 = stg.tile([P, DC, D], f16, tag="Wt2")
                nc.gpsimd.dma_start(
                    encN[:], enc_d.rearrange("(n p) d -> p n d", p=P)
                )
                nc.gpsimd.dma_start(
                    Wt1[:], w_d[0:D, :].rearrange("(r p) d -> p r d", p=P)
                )
                nc.gpsimd.dma_start(
                    Wt2[:], w_d[D : 2 * D, :].rearrange("(r p) d -> p r d", p=P)
                )
                nc.gpsimd.dma_start(scrE[:], enc_d[:])
                nc.gpsimd.dma_start(scrD[:], dec_d[:])
                return dict(scrE=scrE, scrD=scrD, encN=encN, Wt1=Wt1, Wt2=Wt2)

            def stage_transposes(g):
                encT = stg.tile([P, DC, S], f16, tag="encT")
                decT = stg.tile([P, DC, S], f16, tag="decT")
                for src, dst in ((g["scrE"], encT), (g["scrD"], decT)):
                    for dc in range(DC):
                        nc.sync.dma_start(
                            out=dst[:, dc, :],
                            in_=src[:, dc * P : (dc + 1) * P],
                            transpose=True,
                        )
                g["encT"] = encT
                g["decT"] = decT

            def emit_fold(g, b):
                """cross-partition fold of block b's denom partial + recip."""
                eb = g["esumBs"][b]
                pd, rden = g["pd"], g["rden"]
                for j in range(4):
                    c = b * 4 + j
                    nc.tensor.matmul(
                        pd[:, c : c + 1],
                        eb[:, j * P : (j + 1) * P],
                        ones[:],
                        start=(j == 0),
                        stop=(j == 3),
                    )
                nc.vector.reciprocal(
                    rden[:, b * 4 : (b + 1) * 4], pd[:, b * 4 : (b + 1) * 4]
                )

            def phase3_step(gp, c):
                """One output chunk of phase 3 for rep gp (the previous rep
                when interleaved): out[c] = tanh(U[:,c]/den + dec[c] @ W2)."""
                U, rden = gp["U"], gp["rden"]
                decT, Wt1, Wt2 = gp["decT"], gp["Wt1"], gp["Wt2"]
                if c == 1:
                    emit_fold(gp, NSB - 1)
                if c % 4 == 0:
                    gp["outO"] = wrk.tile([P, 4, D], f32, tag="outO", name=f"outO{c}")
                outO = gp["outO"]
                Y = psY.tile([P, 2 * D], f32, tag="Y")
                y1 = Y[:, 0:D]
                y2 = Y[:, D : 2 * D]
                for dc in range(DC):
                    nc.tensor.matmul(
                        y1,
                        U[:, dc, c * P : (c + 1) * P],
                        Wt1[:, dc, :],
                        start=(dc == 0),
                        stop=False,
                    )
                for dc in range(DC):
                    nc.tensor.matmul(
                        y2,
                        decT[:, dc, c * P : (c + 1) * P],
                        Wt2[:, dc, :],
                        start=False,
                        stop=(dc == DC - 1),
                    )
                t1 = wrk.tile([P, D], f32, tag="t1", bufs=4)
                nc.vector.tensor_scalar_mul(t1[:], y1, rden[:, c : c + 1])
                t2 = wrk.tile([P, D], f32, tag="t2", bufs=4)
                nc.vector.tensor_add(t2[:], t1[:], y2)
                nc.scalar.activation(outO[:, c % 4, :], t2[:], AF.Tanh)
                if c % 4 == 3:
                    q = c // 4
                    nc.sync.dma_start(out_v[:, q * 4 : (q + 1) * 4, :], outO[:])

            def body(g, prev):
                """Blocks of rep g, with phase 3 of rep `prev` interleaved
                into block 0's t-loop (one chunk per slot)."""
                encT, decT, encN = g["encT"], g["decT"], g["encN"]

                U = wrk.tile([P, DC, S], bf16, tag="U")
                rden = wrk.tile([P, NT], f32, tag="rden")
                pd = psA.tile([P, NT], f32, tag="pd", bufs=1)
                g["U"], g["rden"], g["pd"] = U, rden, pd
                g["esumBs"] = [None] * NSB

                for b in range(NSB):
                    s_lo = b * SB
                    E = wrk.tile([P, NT, SB], bf16, tag="E")
                    eB = wrk.tile([P, SB], bf16, tag="eB")
                    pu = [
                        psU.tile([P, SB], f32, tag="pu", name=f"pu{b}_{i}")
                        for i in range(DC)
                    ]

                    def ctx_mms(t):
                        for dc in range(DC):
                            nc.tensor.matmul(
                                pu[dc][:],
                                encN[:, t, dc * P : (dc + 1) * P],
                                E[:, t, :],
                                start=(t == 0),
                                stop=(t == NT - 1),
                            )

                    for t in range(NT):
                        ps = psA.tile([P, SB], f32, tag="ps")
                        for dc in range(DC):
                            nc.tensor.matmul(
                                ps[:],
                                decT[:, dc, t * P : (t + 1) * P],
                                encT[:, dc, s_lo : s_lo + SB],
                                start=(dc == 0),
                                stop=(dc == DC - 1),
                            )
                        nc.scalar.activation(
                            E[:, t, :], ps[:], AF.Exp, bias=nshift[:]
                        )
                        if t >= 1:
                            ctx_mms(t - 1)
                        # serial denom accumulation, one add per slot
                        if t == 1:
                            nc.vector.tensor_add(eB[:], E[:, 0, :], E[:, 1, :])
                        elif t >= 2:
                            nc.vector.tensor_add(eB[:], eB[:], E[:, t, :])
                        if b <= 1 and prev is not None:
                            slot = b * NT + t
                            if slot % 2 == 0:
                                phase3_step(prev, slot // 2)
                        if b >= 1 and t == 8:
                            emit_fold(g, b - 1)
                        if b == 1 and t == 4 and g.get("nxt") is not None:
                            stage_transposes(g["nxt"])
                    ctx_mms(NT - 1)
                    for dc in range(DC):
                        nc.vector.tensor_copy(U[:, dc, s_lo : s_lo + SB], pu[dc][:])
                    g["esumBs"][b] = eB

            cur = stage_casts()
            stage_transposes(cur)
            prev = None
            for r in range(reps):
                nxt = stage_casts() if r + 1 < reps else None
                cur["nxt"] = nxt
                body(cur, prev)
                prev = cur
                cur = nxt
            # tail: phase 3 of the last rep
            for c in range(NT):
                phase3_step(prev, c)

    nc.compile()
    return nc


def get_nc():
    if "nc" not in _CACHE:
        _CACHE["nc"] = _build()
    return _CACHE["nc"]


def _get_fn():
    """Build the sharded PJRT executable once and cache it; subsequent
    kernel() calls pay only input transfer + dispatch."""
    if "fn" in _CACHE:
        return _CACHE["fn"]
    import jax
    from jax.sharding import Mesh, NamedSharding, PartitionSpec
    from jax.experimental.shard_map import shard_map
    from concourse.bass2jax import (
        _bass_exec_p,
        install_neuronx_cc_hook,
        partition_id_tensor,
    )

    install_neuronx_cc_hook()
    nc = get_nc()
    out_avals = []
    for alloc in nc.m.functions[0].allocations:
        if (
            isinstance(alloc, mybir.MemoryLocationSet)
            and alloc.kind == "ExternalOutput"
        ):
            out_avals.append(
                jax.core.ShapedArray(
                    tuple(alloc.tensor_shape), mybir.dt.np(alloc.dtype)
                )
            )
    has_pid = nc.partition_id_tensor is not None
    names = ["enc", "dec", "w", "out"] + (["partition_id"] if has_pid else [])
    mesh = Mesh(np.asarray(jax.devices()[:B]), ("core",))
    spec = PartitionSpec("core")

    def _b(e, d, ww, z):
        ops = [e, d, ww, z] + ([partition_id_tensor()] if has_pid else [])
        return _bass_exec_p.bind(
            *ops,
            out_avals=tuple(out_avals),
            in_names=tuple(names),
            out_names=("out",),
            lowering_input_output_aliases=(),
            sim_require_finite=True,
            sim_require_nnan=True,
            nc=nc,
        )[0]

    jitted = jax.jit(
        shard_map(
            _b, mesh=mesh, in_specs=(spec,) * 4, out_specs=spec, check_rep=False
        ),
        donate_argnums=(3,),
        keep_unused=True,
    )
    sh = NamedSharding(mesh, spec)
    _CACHE["fn"] = (jitted, sh)
    return _CACHE["fn"]


def kernel(enc_outputs_top, dec_outputs_top, W_tanh):
    import jax

    enc = np.ascontiguousarray(enc_outputs_top, dtype=np.float32)
    dec = np.ascontiguousarray(dec_outputs_top, dtype=np.float32)
    w = np.ascontiguousarray(W_tanh, dtype=np.float32)
    try:
        fn, sh = _get_fn()
        eg = jax.device_put(enc.reshape(B * S, D), sh)
        dg = jax.device_put(dec.reshape(B * S, D), sh)
        wg = jax.device_put(np.concatenate([w] * B, axis=0), sh)
        zg = jax.device_put(np.zeros((B * S, D), np.float32), sh)
        out = np.asarray(jax.block_until_ready(fn(eg, dg, wg, zg)))
        return out.reshape(B, S, D)
    except Exception:
        # fallback: reference multi-core path (rebuilds the jit per call)
        nc = get_nc()
        in_maps = [{"enc": enc[b], "dec": dec[b], "w": w} for b in range(B)]
        res = bass_utils.run_bass_kernel_spmd(nc, in_maps, core_ids=list(range(B)))
        return np.stack([r["out"] for r in res.results], axis=0)


# revision 5
# speedup vs baseline: 3.4391x; 1.2196x over previous
"""Luong attention kernel for Trainium2 (Bass/Tile), data-parallel over batch.

Math (per batch b):
    scores[s,t] = enc[s,:] . dec[t,:]
    weights     = softmax(scores, axis=t)
    context[s]  = sum_t weights[s,t] * enc[t,:]
    out         = tanh(concat([context, dec]) @ W_tanh)

v5 design notes:
  - B=8 batches -> 8 NeuronCores, one batch per core, no collectives.
  - Input staging prefetched one rep ahead; staging tiles triple
    buffered so next-rep cast DMAs never wait on PE progress; f32->16bit
    casts ride SWDGE DMAs; transposes via DRAM bounce + xbar.
  - Phases 1+2 software-pipelined per t-chunk: PE slot = [2 scores MMs
    (t), 2 context MMs (t-1)]; the Exp activation (611ns) hides under
    the 864ns of 4 matmuls.
  - softmax via global shift (E = exp(s-64), bf16, unnormalized);
    normalization applied after the final matmul per output row.
  - denominator partials accumulate SERIALLY in bf16 on DVE (one add
    per t-slot, pipelined behind the exps) so each block's partial is
    final one slot after its last exp; the cross-partition 128-fold is
    a single 4-matmul group per block placed half a block later.
  - PHASE 3 OF REP r IS INTERLEAVED INTO REP r+1's BLOCK-0 t-LOOP (one
    output chunk per t-slot): its 4 matmuls fill the PE slack behind
    block-0's own matmuls, removing both the per-chunk DVE-paced PE
    bubbles and the rep-boundary drain.  The last rep runs phase 3 as
    a tail.
  - PSUM: 3 (scores) + 1 (pd) + 2 (context) + 2 (y1/y2) = 8 banks;
    phase-3 y1/y2 share one bank as a single 4-matmul group.
"""

import sys

if "/opt/trn_rl_repo" not in sys.path:
    sys.path.insert(0, "/opt/trn_rl_repo")

import numpy as np

import concourse.bacc as bacc
import concourse.mybir as mybir
import concourse.tile as tile
from concourse import bass_utils

B, S, D = 8, 2048, 256
P = 128
NT = S // P
SB = 512
NSB = S // SB
DC = D // P
SHIFT = 64.0

_CACHE = {}


def _build(reps: int = 1):
    f32, bf16, f16 = mybir.dt.float32, mybir.dt.bfloat16, mybir.dt.float16
    AF = mybir.ActivationFunctionType

    nc = bacc.Bacc("TRN2", target_bir_lowering=False, debug=False)
    enc_d = nc.dram_tensor("enc", [S, D], f32, kind="ExternalInput").ap()
    dec_d = nc.dram_tensor("dec", [S, D], f32, kind="ExternalInput").ap()
    w_d = nc.dram_tensor("w", [2 * D, D], f32, kind="ExternalInput").ap()
    out_d = nc.dram_tensor("out", [S, D], f32, kind="ExternalOutput").ap()
    out_v = out_d.rearrange("(n p) d -> p n d", p=P)

    with tile.TileContext(nc) as tc:
        with (
            tc.tile_pool(name="pers", bufs=1) as pers,
            tc.tile_pool(name="stg", bufs=3) as stg,
            tc.tile_pool(name="scr", bufs=2, space="DRAM") as scr,
            tc.tile_pool(name="wrk", bufs=2) as wrk,
            tc.tile_pool(name="psA", bufs=3, space="PSUM") as psA,
            tc.tile_pool(name="psU", bufs=2, space="PSUM") as psU,
            tc.tile_pool(name="psY", bufs=2, space="PSUM") as psY,
        ):
            ones = pers.tile([P, 1], bf16, tag="ones")
            nshift = pers.tile([P, 1], f32, tag="nshift")
            nc.any.memset(ones[:], 1.0)
            nc.any.memset(nshift[:], -SHIFT)

            def stage_casts():
                scrE = scr.tile([S, D], f16, tag="scrE")
                scrD = scr.tile([S, D], f16, tag="scrD")
                encN = stg.tile([P, NT, D], bf16, tag="encN")
                Wt1 = stg.tile([P, DC, D], bf16, tag="Wt1")
                Wt2 = stg.tile([P, DC, D], f16, tag="Wt2")
                nc.gpsimd.dma_start(
                    Wt1[:], w_d[0:D, :].rearrange("(r p) d -> p r d", p=P)
                )
                nc.gpsimd.dma_start(
                    Wt2[:], w_d[D : 2 * D, :].rearrange("(r p) d -> p r d", p=P)
                )
                nc.gpsimd.dma_start(scrE[:], enc_d[:])
                nc.gpsimd.dma_start(scrD[:], dec_d[:])
                # encN from the f16 bounce (1MB read instead of 2MB); emitted
                # last so its wait on scrE only idles the Pool queue tail
                nc.gpsimd.dma_start(
                    encN[:], scrE.rearrange("(n p) d -> p n d", p=P)
                )
                return dict(scrE=scrE, scrD=scrD, encN=encN, Wt1=Wt1, Wt2=Wt2)

            def stage_transposes(g):
                encT = stg.tile([P, DC, S], f16, tag="encT")
                decT = stg.tile([P, DC, S], f16, tag="decT")
                for src, dst in ((g["scrE"], encT), (g["scrD"], decT)):
                    for dc in range(DC):
                        nc.sync.dma_start(
                            out=dst[:, dc, :],
                            in_=src[:, dc * P : (dc + 1) * P],
                            transpose=True,
                        )
                g["encT"] = encT
                g["decT"] = decT

            def emit_fold(g, b):
                """cross-partition fold of block b's denom partial + recip."""
                eb = g["esumBs"][b]
                pd, rden = g["pd"], g["rden"]
                for j in range(4):
                    c = b * 4 + j
                    nc.tensor.matmul(
                        pd[:, c : c + 1],
                        eb[:, j * P : (j + 1) * P],
                        ones[:],
                        start=(j == 0),
                        stop=(j == 3),
                    )
                nc.vector.reciprocal(
                    rden[:, b * 4 : (b + 1) * 4], pd[:, b * 4 : (b + 1) * 4]
                )

            def phase3_step(gp, c):
                """One output chunk of phase 3 for rep gp (the previous rep
                when interleaved): out[c] = tanh(U[:,c]/den + dec[c] @ W2)."""
                U, rden = gp["U"], gp["rden"]
                decT, Wt1, Wt2 = gp["decT"], gp["Wt1"], gp["Wt2"]
                if c == 1:
                    emit_fold(gp, NSB - 1)
                if c % 4 == 0:
                    gp["outO"] = wrk.tile([P, 4, D], f32, tag="outO", name=f"outO{c}")
                outO = gp["outO"]
                Y = psY.tile([P, 2 * D], f32, tag="Y")
                y1 = Y[:, 0:D]
                y2 = Y[:, D : 2 * D]
                for dc in range(DC):
                    nc.tensor.matmul(
                        y1,
                        U[:, dc, c * P : (c + 1) * P],
                        Wt1[:, dc, :],
                        start=(dc == 0),
                        stop=False,
                    )
                for dc in range(DC):
                    nc.tensor.matmul(
                        y2,
                        decT[:, dc, c * P : (c + 1) * P],
                        Wt2[:, dc, :],
                        start=False,
                        stop=(dc == DC - 1),
                    )
                t1 = wrk.tile([P, D], f32, tag="t1", bufs=4)
                nc.vector.tensor_scalar_mul(t1[:], y1, rden[:, c : c + 1])
                t2 = wrk.tile([P, D], f32, tag="t2", bufs=4)
                nc.vector.tensor_add(t2[:], t1[:], y2)
                nc.scalar.activation(outO[:, c % 4, :], t2[:], AF.Tanh)
                if c % 4 == 3:
                    q = c // 4
                    nc.sync.dma_start(out_v[:, q * 4 : (q + 1) * 4, :], outO[:])

            def body(g, prev):
                """Blocks of rep g, with phase 3 of rep `prev` interleaved
                into block 0's t-loop (one chunk per slot)."""
                encT, decT, encN = g["encT"], g["decT"], g["encN"]

                U = wrk.tile([P, DC, S], bf16, tag="U")
                rden = wrk.tile([P, NT], f32, tag="rden")
                pd = psA.tile([P, NT], f32, tag="pd", bufs=1)
                g["U"], g["rden"], g["pd"] = U, rden, pd
                g["esumBs"] = [None] * NSB

                for b in range(NSB):
                    s_lo = b * SB
                    E = wrk.tile([P, NT, SB], bf16, tag="E")
                    eB = wrk.tile([P, SB], bf16, tag="eB")
                    pu = [
                        psU.tile([P, SB], f32, tag="pu", name=f"pu{b}_{i}")
                        for i in range(DC)
                    ]

                    def ctx_mms(t):
                        for dc in range(DC):
                            nc.tensor.matmul(
                                pu[dc][:],
                                encN[:, t, dc * P : (dc + 1) * P],
                                E[:, t, :],
                                start=(t == 0),
                                stop=(t == NT - 1),
                            )

                    for t in range(NT):
                        ps = psA.tile([P, SB], f32, tag="ps")
                        for dc in range(DC):
                            nc.tensor.matmul(
                                ps[:],
                                decT[:, dc, t * P : (t + 1) * P],
                                encT[:, dc, s_lo : s_lo + SB],
                                start=(dc == 0),
                                stop=(dc == DC - 1),
                            )
                        nc.scalar.activation(
                            E[:, t, :], ps[:], AF.Exp, bias=nshift[:]
                        )
                        if t >= 1:
                            ctx_mms(t - 1)
                        # serial denom accumulation, one add per slot
                        if t == 1:
                            nc.vector.tensor_add(eB[:], E[:, 0, :], E[:, 1, :])
                        elif t >= 2:
                            nc.vector.tensor_add(eB[:], eB[:], E[:, t, :])
                        if b <= 1 and prev is not None:
                            slot = b * NT + t
                            if slot % 2 == 0:
                                phase3_step(prev, slot // 2)
                        if b >= 1 and t == 8:
                            emit_fold(g, b - 1)
                        if b == 1 and t == 4 and g.get("nxt") is not None:
                            stage_transposes(g["nxt"])
                    ctx_mms(NT - 1)
                    for dc in range(DC):
                        nc.vector.tensor_copy(U[:, dc, s_lo : s_lo + SB], pu[dc][:])
                    g["esumBs"][b] = eB

            cur = stage_casts()
            stage_transposes(cur)
            prev = None
            for r in range(reps):
                nxt = stage_casts() if r + 1 < reps else None
                cur["nxt"] = nxt
                body(cur, prev)
                prev = cur
                cur = nxt
            # tail: phase 3 of the last rep
            for c in range(NT):
                phase3_step(prev, c)

    nc.compile()
    return nc


def get_nc():
    if "nc" not in _CACHE:
        _CACHE["nc"] = _build()
    return _CACHE["nc"]


def _get_fn():
    """Build the sharded PJRT executable once and cache it; subsequent
    kernel() calls pay only input transfer + dispatch."""
    if "fn" in _CACHE:
        return _CACHE["fn"]
    import jax
    from jax.sharding import Mesh, NamedSharding, PartitionSpec
    from jax.experimental.shard_map import shard_map
    from concourse.bass2jax import (
        _bass_exec_p,
        install_neuronx_cc_hook,
        partition_id_tensor,
    )

    install_neuronx_cc_hook()
    nc = get_nc()
    out_avals = []
    for alloc in nc.m.functions[0].allocations:
        if (
            isinstance(alloc, mybir.MemoryLocationSet)
            and alloc.kind == "ExternalOutput"
        ):
            out_avals.append(
                jax.core.ShapedArray(
                    tuple(alloc.tensor_shape), mybir.dt.np(alloc.dtype)
                )
            )
    has_pid = nc.partition_id_tensor is not None
    names = ["enc", "dec", "w", "out"] + (["partition_id"] if has_pid else [])
    mesh = Mesh(np.asarray(jax.devices()[:B]), ("core",))
    spec = PartitionSpec("core")

    def _b(e, d, ww, z):
        ops = [e, d, ww, z] + ([partition_id_tensor()] if has_pid else [])
        return _bass_exec_p.bind(
            *ops,
            out_avals=tuple(out_avals),
            in_names=tuple(names),
            out_names=("out",),
            lowering_input_output_aliases=(),
            sim_require_finite=True,
            sim_require_nnan=True,
            nc=nc,
        )[0]

    jitted = jax.jit(
        shard_map(
            _b, mesh=mesh, in_specs=(spec,) * 4, out_specs=spec, check_rep=False
        ),
        donate_argnums=(3,),
        keep_unused=True,
    )
    sh = NamedSharding(mesh, spec)
    _CACHE["fn"] = (jitted, sh)
    return _CACHE["fn"]


def kernel(enc_outputs_top, dec_outputs_top, W_tanh):
    import jax

    enc = np.ascontiguousarray(enc_outputs_top, dtype=np.float32)
    dec = np.ascontiguousarray(dec_outputs_top, dtype=np.float32)
    w = np.ascontiguousarray(W_tanh, dtype=np.float32)
    try:
        fn, sh = _get_fn()
        eg = jax.device_put(enc.reshape(B * S, D), sh)
        dg = jax.device_put(dec.reshape(B * S, D), sh)
        wg = jax.device_put(np.concatenate([w] * B, axis=0), sh)
        zg = jax.device_put(np.zeros((B * S, D), np.float32), sh)
        out = np.asarray(jax.block_until_ready(fn(eg, dg, wg, zg)))
        return out.reshape(B, S, D)
    except Exception:
        # fallback: reference multi-core path (rebuilds the jit per call)
        nc = get_nc()
        in_maps = [{"enc": enc[b], "dec": dec[b], "w": w} for b in range(B)]
        res = bass_utils.run_bass_kernel_spmd(nc, in_maps, core_ids=list(range(B)))
        return np.stack([r["out"] for r in res.results], axis=0)
